# revision 9
# baseline (speedup 1.0000x reference)
"""Causal full-d_model attention (B=4, T=2048, C=1024) on 8 Trainium2 cores.

Sharding: core = 2*b + p handles batch b and two 512-row sequence blocks,
p=0 -> blocks {0, 3}, p=1 -> blocks {1, 2} (pairing balances causal work).
K/V projections are split across the pair: each core projects its own
1024-token half (per-core input data places the right half in the same
program slot), then an intra-pair AllGather ([[0,1],[2,3],[4,5],[6,7]] -
the valid LNC1_4x2 shape) assembles the full kT / v, read back into SBUF
in global token order.  This halves the projection matmul work vs
computing K/V redundantly on both cores.

All matmul operands are bf16 (same PE rate as f32r, but FWL halves
LDWEIGHTS and every DMA byte count halves).  Accumulation is fp32 in
PSUM; softmax denominator / reciprocal / biases stay fp32.

On-device layout is transposed ([feature, token]) so every matmul
contracts along the partition axis:
    kT/qT = W.T @ xT            (projection)
    v     = xT.T @ Wv           (token-partition layout)
    scoresT[j, i] = kT_j.T @ qT (j on partitions)
    attnT[c, i]  += v_j.T @ probsT
    outT          = Wo_slice.T @ attnT
Softmax is unnormalized (scores ~ N(0,1), exp is safe); the denominator
comes from an M=1 ones-column matmul over masked exp tiles and is applied
at the PSUM->SBUF copy via a DRAM-broadcast reciprocal row.  Causal masks
arrive as per-core input data, so all 8 cores run one SPMD program.

Input loads ride the sync HWDGE FIFO in exact need-order (first matmul
needs only ~1.3 MB); the AllGather staging/readback path uses the gpsimd
queue so it never blocks input streaming.  Phase-2 emission order hides
the reciprocal round-trip (scores-B between PV-A and O-A) and PV
accumulates in two 4-bank PSUM groups to fit the 8 banks.
"""

import math

import numpy as np

P = 128          # SBUF partitions
B_, T_, C_ = 4, 2048, 1024
RG = [[0, 1], [2, 3], [4, 5], [6, 7]]   # intra-pair replica groups


def _emit(nc, tc, aps, T, C):
    import concourse.bass as bass
    from concourse import mybir
    from concourse.tile_rust import add_dep_helper
    from contextlib import ExitStack

    AFT = mybir.ActivationFunctionType
    f32 = mybir.dt.float32
    bf16 = mybir.dt.bfloat16

    NT = C // P            # feature tiles (8)
    BLK = T // 4           # sequence block = i-slot width FB (512)
    TL = 2 * BLK           # local query tokens per core
    TH = T // 2            # K/V tokens projected per core
    FB = BLK               # matmul moving free dim
    NCHL = TH // BLK       # local K/V chunks (2)
    njA = (2 * BLK) // P   # 8  j-tiles for slot A
    njB = (4 * BLK) // P   # 16 j-tiles for slot B
    CA = P * (njA - 1)
    NJB0 = njA             # slot-B j-tiles < NJB0 are causally all-ones
    CB = P * (njB - 1 - NJB0)
    SC = 1.0 / math.sqrt(C)

    (xT, xq, Wk, Wq, Wv, Wo, bq_t, bk_t, bo_t, ones_d, mA, mB,
     rec_dram, outT) = aps

    with ExitStack() as ctx:
        singles = ctx.enter_context(tc.tile_pool(name="singles", bufs=1))
        kpool = ctx.enter_context(tc.tile_pool(name="kpool", bufs=1))
        qpool = ctx.enter_context(tc.tile_pool(name="qpool", bufs=1))
        vpool = ctx.enter_context(tc.tile_pool(name="vpool", bufs=1))
        wopool = ctx.enter_context(tc.tile_pool(name="wopool", bufs=1))
        psp = ctx.enter_context(tc.tile_pool(name="psp", bufs=8, space="PSUM"))
        dramp = ctx.enter_context(tc.tile_pool(name="dramp", bufs=1,
                                               space="DRAM"))

        bq_sb = singles.tile([P, NT], f32, name="bq_sb")
        bk_sb = singles.tile([P, NT], f32, name="bk_sb")
        bo_sb = singles.tile([P, NT], f32, name="bo_sb")
        ones_sb = singles.tile([P, 1], bf16, name="ones_sb")
        mA_sb = singles.tile([P, CA + FB], bf16, name="mA_sb")
        mB_sb = singles.tile([P, CB + FB], bf16, name="mB_sb")
        nc.sync.dma_start(out=bq_sb, in_=bq_t)
        nc.sync.dma_start(out=bk_sb, in_=bk_t)
        nc.sync.dma_start(out=bo_sb, in_=bo_t)
        nc.sync.dma_start(out=ones_sb, in_=ones_d)

        kT_sb = kpool.tile([P, NT, T], bf16, name="kT_sb")
        qT_sb = qpool.tile([P, NT, TL], bf16, name="qT_sb")
        v_sb = vpool.tile([P, T // P, C], bf16, name="v_sb")
        wo_sb = wopool.tile([P, NT, NT, P], bf16, name="wo_sb")

        # DRAM bounce buffers for the intra-pair AllGather
        kag_in = dramp.tile([P, NT, TH], bf16, name="kag_in")
        kag_out = dramp.tile([2, P, NT, TH], bf16, name="kag_out")
        vag_in = dramp.tile([P, TH // P, C], bf16, name="vag_in")
        vag_out = dramp.tile([2, P, TH // P, C], bf16, name="vag_out")

        # ------- phase 1: K-half, V-half (+AllGathers), then Q -------
        with ExitStack() as p1:
            wpool = p1.enter_context(tc.tile_pool(name="wpool", bufs=1))
            xcpool = p1.enter_context(tc.tile_pool(name="xcpool", bufs=1))
            xqpool = p1.enter_context(tc.tile_pool(name="xqpool", bufs=1))
            kstp = p1.enter_context(tc.tile_pool(name="kstp", bufs=1))
            vstp = p1.enter_context(tc.tile_pool(name="vstp", bufs=6))

            wk_sb = wpool.tile([P, NT, NT, P], bf16, name="wk_sb")
            wq_sb = wpool.tile([P, NT, NT, P], bf16, name="wq_sb")
            wv_sb = wpool.tile([P, NT, C], bf16, name="wv_sb")
            xc_sb = xcpool.tile([P, NCHL, NT, BLK], bf16, name="xc_sb")
            xq_sb = xqpool.tile([P, 2, NT, FB], bf16, name="xq_sb")

            # input loads on the sync FIFO in exact need-order
            nc.sync.dma_start(out=wk_sb[:, 0], in_=Wk[:, 0])
            nc.sync.dma_start(out=xc_sb[:, 0], in_=xT[:, 0])
            for co in range(1, NT):
                nc.sync.dma_start(out=wk_sb[:, co], in_=Wk[:, co])
            nc.sync.dma_start(out=xc_sb[:, 1], in_=xT[:, 1])
            nc.sync.dma_start(out=wv_sb, in_=Wv)
            nc.sync.dma_start(out=xq_sb, in_=xq)
            for co in range(NT):
                nc.sync.dma_start(out=wq_sb[:, co], in_=Wq[:, co])
            nc.sync.dma_start(out=mA_sb, in_=mA)
            nc.sync.dma_start(out=mB_sb, in_=mB)
            nc.sync.dma_start(out=wo_sb, in_=Wo)

            # K-half: kT_loc = Wk.T @ x_half (+bk), staged to DRAM
            for l in range(NCHL):
                kst = kstp.tile([P, NT, BLK], bf16, name="kst", tag="kst")
                for co in range(NT):
                    ps = psp.tile([P, BLK], f32, name="ps_k", tag="ps")
                    for ci in range(NT):
                        nc.tensor.matmul(
                            ps,
                            wk_sb[:, co, ci, :],
                            xc_sb[:, l, ci, :],
                            start=(ci == 0),
                            stop=(ci == NT - 1),
                        )
                    nc.scalar.activation(
                        out=kst[:, co, :], in_=ps, func=AFT.Identity,
                        bias=bk_sb[:, co:co + 1],
                    )
                nc.sync.dma_start(
                    out=kag_in[:, :, l * BLK:(l + 1) * BLK], in_=kst
                )
            nc.gpsimd.collective_compute(
                "AllGather",
                mybir.AluOpType.bypass,
                replica_groups=RG,
                ins=[kag_in.opt()],
                outs=[kag_out.opt()],
            )

            # V-half: v_loc = x_half @ Wv, staged to DRAM
            # (bv is folded into bo_t on the host)
            for l in range(NCHL):
                for jt in range(BLK // P):
                    for ch in range(C // 512):
                        ps = psp.tile([P, 512], f32, name="ps_v", tag="ps")
                        for ci in range(NT):
                            nc.tensor.matmul(
                                ps,
                                xc_sb[:, l, ci, jt * P:(jt + 1) * P],
                                wv_sb[:, ci, ch * 512:(ch + 1) * 512],
                                start=(ci == 0),
                                stop=(ci == NT - 1),
                            )
                        vs = vstp.tile([P, 512], bf16, name="vs", tag="vs")
                        nc.vector.tensor_copy(vs, ps)
                        nc.sync.dma_start(
                            out=vag_in[:, l * (BLK // P) + jt,
                                       ch * 512:(ch + 1) * 512],
                            in_=vs,
                        )
            nc.gpsimd.collective_compute(
                "AllGather",
                mybir.AluOpType.bypass,
                replica_groups=RG,
                ins=[vag_in.opt()],
                outs=[vag_out.opt()],
            )

            # readbacks in global token order (h = rank in pair)
            for h in range(2):
                nc.gpsimd.dma_start(
                    out=kT_sb[:, :, h * TH:(h + 1) * TH], in_=kag_out[h]
                )
            for h in range(2):
                nc.gpsimd.dma_start(
                    out=v_sb[:, h * (TH // P):(h + 1) * (TH // P), :],
                    in_=vag_out[h],
                )

            # Q: qT = Wq.T @ xq (+bq) for the two local 512-blocks
            for s in range(2):
                for co in range(NT):
                    ps = psp.tile([P, FB], f32, name="ps_q", tag="ps")
                    for ci in range(NT):
                        nc.tensor.matmul(
                            ps,
                            wq_sb[:, co, ci, :],
                            xq_sb[:, s, ci, :],
                            start=(ci == 0),
                            stop=(ci == NT - 1),
                        )
                    nc.scalar.activation(
                        out=qT_sb[:, co, s * FB:(s + 1) * FB],
                        in_=ps,
                        func=AFT.Identity,
                        bias=bq_sb[:, co:co + 1],
                    )

        # -------- phase 2: attention + output projection --------
        with ExitStack() as p2:
            probsp = p2.enter_context(tc.tile_pool(name="probsp", bufs=njB))
            attnp = p2.enter_context(tc.tile_pool(name="attnp", bufs=2))
            recp = p2.enter_context(tc.tile_pool(name="recp", bufs=2))
            ostagep = p2.enter_context(tc.tile_pool(name="ostagep", bufs=2))

            SLOTS = [(njA, 0, CA, mA_sb), (njB, NJB0, CB, mB_sb)]

            def emit_scores(a):
                nj, j0m, Cm, m_sb = SLOTS[a]
                pjs = []
                ps_den = psp.tile([1, FB], f32, name="ps_den", tag="ps")
                for jt in range(nj):
                    ps_s = psp.tile([P, FB], f32, name="ps_s", tag="ps")
                    for ci in range(NT):
                        nc.tensor.matmul(
                            ps_s,
                            kT_sb[:, ci, jt * P:(jt + 1) * P],
                            qT_sb[:, ci, a * FB:(a + 1) * FB],
                            start=(ci == 0),
                            stop=(ci == NT - 1),
                        )
                    pj = probsp.tile([P, FB], bf16, name="pj", tag="pj")
                    nc.scalar.activation(out=pj, in_=ps_s, func=AFT.Exp, scale=SC)
                    if jt >= j0m:  # earlier j-tiles are all-ones on every core
                        s0 = Cm - P * (jt - j0m)
                        nc.vector.tensor_mul(pj, pj, m_sb[:, s0:s0 + FB])
                    nc.tensor.matmul(
                        ps_den,
                        ones_sb,
                        pj,
                        start=(jt == 0),
                        stop=(jt == nj - 1),
                        skip_group_check=True,
                    )
                    pjs.append(pj)
                # 1/denominator: quick copy releases the PSUM bank, then the
                # slow reciprocal runs off the SBUF copy; broadcast to 128
                # partitions via a stride-0 DRAM read.
                den_sb = recp.tile([1, FB], f32, name="den_sb", tag="den_sb")
                nc.scalar.copy(den_sb, ps_den)
                rrow = recp.tile([1, FB], f32, name="rrow", tag="rrow")
                nc.vector.reciprocal(rrow, den_sb)
                rec_w = nc.sync.dma_start(out=rec_dram[a:a + 1, :], in_=rrow)
                recipB = recp.tile([P, FB], f32, name="recipB", tag="recipB")
                rec_row = rec_dram[a, :]
                rec_bcast = bass.AP(
                    tensor=rec_row.tensor,
                    offset=rec_row.offset,
                    ap=[[0, P]] + [list(d) for d in rec_row.ap],
                )
                rec_r = nc.sync.dma_start(out=recipB, in_=rec_bcast)
                add_dep_helper(rec_r.ins, rec_w.ins, reason="rec_dram RAW")
                return pjs, recipB

            def emit_pv(a, pjs, recipB):
                # PV in two 4-bank PSUM groups so scores of the next slot
                # can run while the first group's banks drain
                nj = SLOTS[a][0]
                attn_sb = attnp.tile([P, NT, FB], bf16, name="attn_sb",
                                     tag="attn")
                for g0 in (0, NT // 2):
                    ps_attn = [
                        psp.tile([P, FB], f32, name="ps_attn", tag="ps")
                        for _ in range(NT // 2)
                    ]
                    for jt in range(nj):
                        for k, ct in enumerate(range(g0, g0 + NT // 2)):
                            nc.tensor.matmul(
                                ps_attn[k],
                                v_sb[:, jt, ct * P:(ct + 1) * P],
                                pjs[jt],
                                start=(jt == 0),
                                stop=(jt == nj - 1),
                                skip_group_check=True,
                            )
                    for k, ct in enumerate(range(g0, g0 + NT // 2)):
                        nc.vector.tensor_mul(
                            attn_sb[:, ct, :], ps_attn[k], recipB
                        )
                return attn_sb

            def emit_oproj(a, attn_sb):
                for co in range(NT):
                    ps_o = psp.tile([P, FB], f32, name="ps_o", tag="ps")
                    for ci in range(NT):
                        nc.tensor.matmul(
                            ps_o,
                            wo_sb[:, co, ci, :],
                            attn_sb[:, ci, :],
                            start=(ci == 0),
                            stop=(ci == NT - 1),
                        )
                    os_ = ostagep.tile([P, FB], f32, name="os_", tag="os")
                    nc.scalar.activation(
                        out=os_, in_=ps_o, func=AFT.Identity,
                        bias=bo_sb[:, co:co + 1],
                    )
                    nc.sync.dma_start(
                        out=outT[co * P:(co + 1) * P, a * FB:(a + 1) * FB],
                        in_=os_,
                    )

            pjs_A, recB_A = emit_scores(0)
            attn_A = emit_pv(0, pjs_A, recB_A)
            pjs_B, recB_B = emit_scores(1)   # hides slot-A reciprocal latency
            emit_oproj(0, attn_A)
            attn_B = emit_pv(1, pjs_B, recB_B)
            emit_oproj(1, attn_B)


def build_program(T=T_, C=C_, num_cores=8):
    """Build and compile the SPMD Bass program."""
    from concourse import bacc, mybir
    import concourse.tile as tile

    f32 = mybir.dt.float32
    bf16 = mybir.dt.bfloat16
    NT = C // P
    BLK = T // 4
    TL = 2 * BLK
    njA = (2 * BLK) // P
    njB = (4 * BLK) // P
    CA = P * (njA - 1)
    CB = P * (njB - 1 - njA)

    nc = bacc.Bacc(
        "TRN2", target_bir_lowering=False, debug=False, num_devices=num_cores
    )
    xT = nc.dram_tensor("xT", [P, 2, NT, BLK], bf16, kind="ExternalInput").ap()
    xq = nc.dram_tensor("xq", [P, 2, NT, BLK], bf16, kind="ExternalInput").ap()
    Wk = nc.dram_tensor("Wk", [P, NT, NT, P], bf16, kind="ExternalInput").ap()
    Wq = nc.dram_tensor("Wq", [P, NT, NT, P], bf16, kind="ExternalInput").ap()
    Wv = nc.dram_tensor("Wv", [P, NT, C], bf16, kind="ExternalInput").ap()
    Wo = nc.dram_tensor("Wo", [P, NT, NT, P], bf16, kind="ExternalInput").ap()
    bq_t = nc.dram_tensor("bq_t", [P, NT], f32, kind="ExternalInput").ap()
    bk_t = nc.dram_tensor("bk_t", [P, NT], f32, kind="ExternalInput").ap()
    bo_t = nc.dram_tensor("bo_t", [P, NT], f32, kind="ExternalInput").ap()
    ones_d = nc.dram_tensor("ones_d", [P, 1], bf16, kind="ExternalInput").ap()
    mA = nc.dram_tensor("mA", [P, CA + BLK], bf16, kind="ExternalInput").ap()
    mB = nc.dram_tensor("mB", [P, CB + BLK], bf16, kind="ExternalInput").ap()
    rec_dram = nc.dram_tensor("rec_int", [2, BLK], f32).ap()
    outT = nc.dram_tensor("outT", [C, TL], f32, kind="ExternalOutput").ap()

    aps = (xT, xq, Wk, Wq, Wv, Wo, bq_t, bk_t, bo_t, ones_d, mA, mB,
           rec_dram, outT)
    with tile.TileContext(nc) as tc:
        _emit(nc, tc, aps, T, C)
    nc.compile()
    return nc


def make_core_inputs(x, Wq, bq, Wk, bk, Wv, bv, Wo, bo, T=T_, C=C_):
    """Per-core input maps (list of 8 dicts) for the SPMD program."""
    import ml_dtypes

    f = np.float32
    bf = ml_dtypes.bfloat16
    NT = C // P
    BLK = T // 4
    njA = (2 * BLK) // P
    njB = (4 * BLK) // P
    CA = P * (njA - 1)
    CB = P * (njB - 1 - njA)

    x = np.asarray(x, f)
    Wq, Wk, Wv, Wo = (np.asarray(w, f) for w in (Wq, Wk, Wv, Wo))
    bq, bk, bv, bo = (np.asarray(b, f) for b in (bq, bk, bv, bo))

    def panels(W):  # [C, C] -> [P, co, ci, m]: W[ci*P+p, co*P+m]
        return np.ascontiguousarray(
            W.reshape(NT, P, NT, P).transpose(1, 2, 0, 3)
        ).astype(bf)

    Wk_t = panels(Wk)
    Wq_t = panels(Wq)
    Wo_t = panels(Wo)
    # [C, C] -> [P, ci, m]: Wv[ci*P+p, m]
    Wv_t = np.ascontiguousarray(
        Wv.reshape(NT, P, C).transpose(1, 0, 2)
    ).astype(bf)
    bo_eff = (bv @ Wo + bo).astype(f)

    def tr(b):  # [C] -> [P, NT] with b_t[p, t] = b[t*P + p]
        return np.ascontiguousarray(b.reshape(NT, P).T)

    def mask(CC, i0, width):
        pp = np.arange(P, dtype=np.int64)[:, None]
        gg = np.arange(width, dtype=np.int64)[None, :]
        return np.ascontiguousarray((pp <= gg - CC + i0).astype(bf))

    ones = np.ones((P, 1), bf)

    maps = []
    for core in range(8):
        b, p = core // 2, core % 2
        lo, hi = (0, 3) if p == 0 else (1, 2)
        # [P, chunk, ci, t'] = x[b, chunk*BLK+t', ci*P+p]
        xTv = np.ascontiguousarray(
            x[b].reshape(4, BLK, NT, P).transpose(3, 0, 2, 1)
        ).astype(bf)
        xhalf = np.ascontiguousarray(xTv[:, [2 * p, 2 * p + 1]])
        xqb = np.ascontiguousarray(xTv[:, [lo, hi]])
        maps.append(
            {
                "xT": xhalf,
                "xq": xqb,
                "Wk": Wk_t,
                "Wq": Wq_t,
                "Wv": Wv_t,
                "Wo": Wo_t,
                "bq_t": tr(bq),
                "bk_t": tr(bk),
                "bo_t": tr(bo_eff),
                "ones_d": ones,
                "mA": mask(CA, lo * BLK, CA + BLK),
                "mB": mask(CB + njA * P, hi * BLK, CB + BLK),
            }
        )
    return maps


def gather_output(results, T=T_, C=C_, B=B_):
    BLK = T // 4
    out = np.empty((B, T, C), np.float32)
    for core in range(8):
        b, p = core // 2, core % 2
        lo, hi = (0, 3) if p == 0 else (1, 2)
        oT = results[core]["outT"]
        out[b, lo * BLK:(lo + 1) * BLK] = oT[:, 0:BLK].T
        out[b, hi * BLK:(hi + 1) * BLK] = oT[:, BLK:2 * BLK].T
    return out


_NC_CACHE = {}


def kernel(x, Wq, bq, Wk, bk, Wv, bv, Wo, bo):
    from concourse.bass_utils import run_bass_kernel_spmd

    key = "full"
    if key not in _NC_CACHE:
        _NC_CACHE[key] = build_program()
    nc = _NC_CACHE[key]
    in_maps = make_core_inputs(x, Wq, bq, Wk, bk, Wv, bv, Wo, bo)
    res = run_bass_kernel_spmd(nc, in_maps, list(range(8))).results
    return gather_output(res)


# revision 13
# speedup vs baseline: 1.0897x; 1.0897x over previous
"""Causal full-d_model attention (B=4, T=2048, C=1024) on 8 Trainium2 cores.

Sharding: core = 2*b + p handles batch b and two 512-row sequence blocks,
p=0 -> blocks {0, 3}, p=1 -> blocks {1, 2} (pairing balances causal work).
K/V projections are split across the pair: each core projects its own
1024-token half (per-core input data places the right half in the same
program slot), then an intra-pair AllGather ([[0,1],[2,3],[4,5],[6,7]] -
the valid LNC1_4x2 shape) assembles the full kT / v, read back into SBUF
in global token order.  This halves the projection matmul work vs
computing K/V redundantly on both cores.

All matmul operands are bf16 (same PE rate as f32r, but FWL halves
LDWEIGHTS and every DMA byte count halves).  Accumulation is fp32 in
PSUM; softmax denominator / reciprocal / biases stay fp32.

On-device layout is transposed ([feature, token]) so every matmul
contracts along the partition axis:
    kT/qT = W.T @ xT            (projection)
    v     = xT.T @ Wv           (token-partition layout)
    scoresT[j, i] = kT_j.T @ qT (j on partitions)
    attnT[c, i]  += v_j.T @ probsT
    outT          = Wo_slice.T @ attnT
Softmax is unnormalized (scores ~ N(0,1), exp is safe); the denominator
comes from an M=1 ones-column matmul over masked exp tiles and is applied
at the PSUM->SBUF copy via a DRAM-broadcast reciprocal row.  Causal masks
arrive as per-core input data, so all 8 cores run one SPMD program.

Input loads ride the sync HWDGE FIFO in exact need-order (first matmul
needs only ~1.3 MB); the AllGather staging/readback path uses the gpsimd
queue so it never blocks input streaming.  Phase-2 emission order hides
the reciprocal round-trip (scores-B between PV-A and O-A) and PV
accumulates in two 4-bank PSUM groups to fit the 8 banks.
"""

import math

import numpy as np

P = 128          # SBUF partitions
B_, T_, C_ = 4, 2048, 1024
RG = [[0, 1], [2, 3], [4, 5], [6, 7]]   # intra-pair replica groups


def _emit(nc, tc, aps, T, C):
    import concourse.bass as bass
    from concourse import mybir
    from concourse.tile_rust import add_dep_helper
    from contextlib import ExitStack

    AFT = mybir.ActivationFunctionType
    f32 = mybir.dt.float32
    bf16 = mybir.dt.bfloat16

    NT = C // P            # feature tiles (8)
    BLK = T // 4           # sequence block = i-slot width FB (512)
    TL = 2 * BLK           # local query tokens per core
    TH = T // 2            # K/V tokens projected per core
    FB = BLK               # matmul moving free dim
    NCHL = TH // BLK       # local K/V chunks (2)
    njA = (2 * BLK) // P   # 8  j-tiles for slot A
    njB = (4 * BLK) // P   # 16 j-tiles for slot B
    CA = P * (njA - 1)
    NJB0 = njA             # slot-B j-tiles < NJB0 are causally all-ones
    CB = P * (njB - 1 - NJB0)
    SC = 1.0 / math.sqrt(C)

    (xT, xq, Wk, Wq, Wv, Wo, bq_t, bk_t, bo_t, ones_d, mA, mB,
     rec_dram, outT) = aps

    with ExitStack() as ctx:
        singles = ctx.enter_context(tc.tile_pool(name="singles", bufs=1))
        kpool = ctx.enter_context(tc.tile_pool(name="kpool", bufs=1))
        qpool = ctx.enter_context(tc.tile_pool(name="qpool", bufs=1))
        vpool = ctx.enter_context(tc.tile_pool(name="vpool", bufs=1))
        wopool = ctx.enter_context(tc.tile_pool(name="wopool", bufs=1))
        psp = ctx.enter_context(tc.tile_pool(name="psp", bufs=8, space="PSUM"))
        dramp = ctx.enter_context(tc.tile_pool(name="dramp", bufs=1,
                                               space="DRAM"))

        bq_sb = singles.tile([P, NT], f32, name="bq_sb")
        bk_sb = singles.tile([P, NT], f32, name="bk_sb")
        bo_sb = singles.tile([P, NT], f32, name="bo_sb")
        ones_sb = singles.tile([P, 1], bf16, name="ones_sb")
        mA_sb = singles.tile([P, CA + FB], bf16, name="mA_sb")
        mB_sb = singles.tile([P, CB + FB], bf16, name="mB_sb")
        nc.sync.dma_start(out=bq_sb, in_=bq_t)
        nc.sync.dma_start(out=bk_sb, in_=bk_t)
        nc.sync.dma_start(out=bo_sb, in_=bo_t)
        nc.sync.dma_start(out=ones_sb, in_=ones_d)

        kT_sb = kpool.tile([P, NT, T], bf16, name="kT_sb")
        qT_sb = qpool.tile([P, NT, TL], bf16, name="qT_sb")
        v_sb = vpool.tile([P, T // P, C], bf16, name="v_sb")
        wo_sb = wopool.tile([P, NT, NT, P], bf16, name="wo_sb")

        # DRAM bounce buffers for the intra-pair AllGather
        kag_in = dramp.tile([P, NT, TH], bf16, name="kag_in")
        kag_out = dramp.tile([2, P, NT, TH], bf16, name="kag_out")
        vag_in = dramp.tile([P, TH // P, C], bf16, name="vag_in")
        vag_out = dramp.tile([2, P, TH // P, C], bf16, name="vag_out")

        # ------- phase 1: K-half, V-half (+AllGathers), then Q -------
        with ExitStack() as p1:
            wpool = p1.enter_context(tc.tile_pool(name="wpool", bufs=1))
            xcpool = p1.enter_context(tc.tile_pool(name="xcpool", bufs=1))
            xqpool = p1.enter_context(tc.tile_pool(name="xqpool", bufs=1))
            kstp = p1.enter_context(tc.tile_pool(name="kstp", bufs=2))
            vstp = p1.enter_context(tc.tile_pool(name="vstp", bufs=4))

            wk_sb = wpool.tile([P, NT, NT, P], bf16, name="wk_sb")
            wq_sb = wpool.tile([P, NT, NT, P], bf16, name="wq_sb")
            wv_sb = wpool.tile([P, NT, C], bf16, name="wv_sb")
            xc_sb = xcpool.tile([P, NCHL, NT, BLK], bf16, name="xc_sb")
            xq_sb = xqpool.tile([P, 2, NT, FB], bf16, name="xq_sb")

            # input loads on the sync FIFO in exact need-order
            nc.sync.dma_start(out=wk_sb[:, 0], in_=Wk[:, 0])
            nc.sync.dma_start(out=xc_sb[:, 0], in_=xT[:, 0])
            for co in range(1, NT):
                nc.sync.dma_start(out=wk_sb[:, co], in_=Wk[:, co])
            nc.sync.dma_start(out=xc_sb[:, 1], in_=xT[:, 1])
            nc.sync.dma_start(out=wv_sb, in_=Wv)
            nc.sync.dma_start(out=xq_sb, in_=xq)
            for co in range(NT):
                nc.sync.dma_start(out=wq_sb[:, co], in_=Wq[:, co])
            nc.sync.dma_start(out=mA_sb, in_=mA)
            nc.sync.dma_start(out=mB_sb, in_=mB)
            nc.sync.dma_start(out=wo_sb, in_=Wo)

            # K-half: kT_loc = Wk.T @ x_half (+bk), staged to DRAM
            for l in range(NCHL):
                kst = kstp.tile([P, NT, BLK], bf16, name="kst", tag="kst")
                for co in range(NT):
                    ps = psp.tile([P, BLK], f32, name="ps_k", tag="ps")
                    for ci in range(NT):
                        nc.tensor.matmul(
                            ps,
                            wk_sb[:, co, ci, :],
                            xc_sb[:, l, ci, :],
                            start=(ci == 0),
                            stop=(ci == NT - 1),
                        )
                    nc.scalar.activation(
                        out=kst[:, co, :], in_=ps, func=AFT.Identity,
                        bias=bk_sb[:, co:co + 1],
                    )
                nc.gpsimd.dma_start(
                    out=kag_in[:, :, l * BLK:(l + 1) * BLK], in_=kst
                )
            nc.gpsimd.collective_compute(
                "AllGather",
                mybir.AluOpType.bypass,
                replica_groups=RG,
                ins=[kag_in.opt()],
                outs=[kag_out.opt()],
            )

            # V-half: v_loc = x_half @ Wv, staged to DRAM
            # (bv is folded into bo_t on the host)
            for l in range(NCHL):
                for jt in range(BLK // P):
                    for ch in range(C // 512):
                        ps = psp.tile([P, 512], f32, name="ps_v", tag="ps")
                        for ci in range(NT):
                            nc.tensor.matmul(
                                ps,
                                xc_sb[:, l, ci, jt * P:(jt + 1) * P],
                                wv_sb[:, ci, ch * 512:(ch + 1) * 512],
                                start=(ci == 0),
                                stop=(ci == NT - 1),
                            )
                        vs = vstp.tile([P, 512], bf16, name="vs", tag="vs")
                        nc.vector.tensor_copy(vs, ps)
                        nc.gpsimd.dma_start(
                            out=vag_in[:, l * (BLK // P) + jt,
                                       ch * 512:(ch + 1) * 512],
                            in_=vs,
                        )
            nc.gpsimd.collective_compute(
                "AllGather",
                mybir.AluOpType.bypass,
                replica_groups=RG,
                ins=[vag_in.opt()],
                outs=[vag_out.opt()],
            )

            # readbacks in global token order (h = rank in pair); on the
            # sync queue so they never delay the AllGather triggers
            for h in range(2):
                nc.sync.dma_start(
                    out=kT_sb[:, :, h * TH:(h + 1) * TH], in_=kag_out[h]
                )
            for h in range(2):
                nc.sync.dma_start(
                    out=v_sb[:, h * (TH // P):(h + 1) * (TH // P), :],
                    in_=vag_out[h],
                )

            # Q: qT = Wq.T @ xq (+bq) for the two local 512-blocks
            for s in range(2):
                for co in range(NT):
                    ps = psp.tile([P, FB], f32, name="ps_q", tag="ps")
                    for ci in range(NT):
                        nc.tensor.matmul(
                            ps,
                            wq_sb[:, co, ci, :],
                            xq_sb[:, s, ci, :],
                            start=(ci == 0),
                            stop=(ci == NT - 1),
                        )
                    nc.scalar.activation(
                        out=qT_sb[:, co, s * FB:(s + 1) * FB],
                        in_=ps,
                        func=AFT.Identity,
                        bias=bq_sb[:, co:co + 1],
                    )

        # -------- phase 2: attention + output projection --------
        with ExitStack() as p2:
            probsp = p2.enter_context(tc.tile_pool(name="probsp", bufs=njB))
            attnp = p2.enter_context(tc.tile_pool(name="attnp", bufs=2))
            recp = p2.enter_context(tc.tile_pool(name="recp", bufs=2))
            ostagep = p2.enter_context(tc.tile_pool(name="ostagep", bufs=2))

            SLOTS = [(njA, 0, CA, mA_sb), (njB, NJB0, CB, mB_sb)]

            def emit_scores(a):
                nj, j0m, Cm, m_sb = SLOTS[a]
                pjs = []
                ps_den = psp.tile([1, FB], f32, name="ps_den", tag="ps")
                for jt in range(nj):
                    ps_s = psp.tile([P, FB], f32, name="ps_s", tag="ps")
                    for ci in range(NT):
                        nc.tensor.matmul(
                            ps_s,
                            kT_sb[:, ci, jt * P:(jt + 1) * P],
                            qT_sb[:, ci, a * FB:(a + 1) * FB],
                            start=(ci == 0),
                            stop=(ci == NT - 1),
                        )
                    pj = probsp.tile([P, FB], bf16, name="pj", tag="pj")
                    nc.scalar.activation(out=pj, in_=ps_s, func=AFT.Exp, scale=SC)
                    if jt >= j0m:  # earlier j-tiles are all-ones on every core
                        s0 = Cm - P * (jt - j0m)
                        nc.vector.tensor_mul(pj, pj, m_sb[:, s0:s0 + FB])
                    nc.tensor.matmul(
                        ps_den,
                        ones_sb,
                        pj,
                        start=(jt == 0),
                        stop=(jt == nj - 1),
                        skip_group_check=True,
                    )
                    pjs.append(pj)
                # 1/denominator: quick copy releases the PSUM bank, then the
                # slow reciprocal runs off the SBUF copy; broadcast to 128
                # partitions via a stride-0 DRAM read.
                den_sb = recp.tile([1, FB], f32, name="den_sb", tag="den_sb")
                nc.scalar.copy(den_sb, ps_den)
                rrow = recp.tile([1, FB], f32, name="rrow", tag="rrow")
                nc.vector.reciprocal(rrow, den_sb)
                rec_w = nc.sync.dma_start(out=rec_dram[a:a + 1, :], in_=rrow)
                recipB = recp.tile([P, FB], f32, name="recipB", tag="recipB")
                rec_row = rec_dram[a, :]
                rec_bcast = bass.AP(
                    tensor=rec_row.tensor,
                    offset=rec_row.offset,
                    ap=[[0, P]] + [list(d) for d in rec_row.ap],
                )
                rec_r = nc.sync.dma_start(out=recipB, in_=rec_bcast)
                add_dep_helper(rec_r.ins, rec_w.ins, reason="rec_dram RAW")
                return pjs, recipB

            def emit_pv(a, pjs, recipB):
                # PV in two 4-bank PSUM groups so scores of the next slot
                # can run while the first group's banks drain
                nj = SLOTS[a][0]
                attn_sb = attnp.tile([P, NT, FB], bf16, name="attn_sb",
                                     tag="attn")
                for g0 in (0, NT // 2):
                    ps_attn = [
                        psp.tile([P, FB], f32, name="ps_attn", tag="ps")
                        for _ in range(NT // 2)
                    ]
                    for jt in range(nj):
                        for k, ct in enumerate(range(g0, g0 + NT // 2)):
                            nc.tensor.matmul(
                                ps_attn[k],
                                v_sb[:, jt, ct * P:(ct + 1) * P],
                                pjs[jt],
                                start=(jt == 0),
                                stop=(jt == nj - 1),
                                skip_group_check=True,
                            )
                    for k, ct in enumerate(range(g0, g0 + NT // 2)):
                        nc.vector.tensor_mul(
                            attn_sb[:, ct, :], ps_attn[k], recipB
                        )
                return attn_sb

            def emit_oproj(a, attn_sb):
                for co in range(NT):
                    ps_o = psp.tile([P, FB], f32, name="ps_o", tag="ps")
                    for ci in range(NT):
                        nc.tensor.matmul(
                            ps_o,
                            wo_sb[:, co, ci, :],
                            attn_sb[:, ci, :],
                            start=(ci == 0),
                            stop=(ci == NT - 1),
                        )
                    os_ = ostagep.tile([P, FB], f32, name="os_", tag="os")
                    nc.scalar.activation(
                        out=os_, in_=ps_o, func=AFT.Identity,
                        bias=bo_sb[:, co:co + 1],
                    )
                    nc.sync.dma_start(
                        out=outT[co * P:(co + 1) * P, a * FB:(a + 1) * FB],
                        in_=os_,
                    )

            pjs_A, recB_A = emit_scores(0)
            attn_A = emit_pv(0, pjs_A, recB_A)
            pjs_B, recB_B = emit_scores(1)   # hides slot-A reciprocal latency
            emit_oproj(0, attn_A)
            attn_B = emit_pv(1, pjs_B, recB_B)
            emit_oproj(1, attn_B)


def build_program(T=T_, C=C_, num_cores=8):
    """Build and compile the SPMD Bass program."""
    from concourse import bacc, mybir
    import concourse.tile as tile

    f32 = mybir.dt.float32
    bf16 = mybir.dt.bfloat16
    NT = C // P
    BLK = T // 4
    TL = 2 * BLK
    njA = (2 * BLK) // P
    njB = (4 * BLK) // P
    CA = P * (njA - 1)
    CB = P * (njB - 1 - njA)

    nc = bacc.Bacc(
        "TRN2", target_bir_lowering=False, debug=False, num_devices=num_cores
    )
    xT = nc.dram_tensor("xT", [P, 2, NT, BLK], bf16, kind="ExternalInput").ap()
    xq = nc.dram_tensor("xq", [P, 2, NT, BLK], bf16, kind="ExternalInput").ap()
    Wk = nc.dram_tensor("Wk", [P, NT, NT, P], bf16, kind="ExternalInput").ap()
    Wq = nc.dram_tensor("Wq", [P, NT, NT, P], bf16, kind="ExternalInput").ap()
    Wv = nc.dram_tensor("Wv", [P, NT, C], bf16, kind="ExternalInput").ap()
    Wo = nc.dram_tensor("Wo", [P, NT, NT, P], bf16, kind="ExternalInput").ap()
    bq_t = nc.dram_tensor("bq_t", [P, NT], f32, kind="ExternalInput").ap()
    bk_t = nc.dram_tensor("bk_t", [P, NT], f32, kind="ExternalInput").ap()
    bo_t = nc.dram_tensor("bo_t", [P, NT], f32, kind="ExternalInput").ap()
    ones_d = nc.dram_tensor("ones_d", [P, 1], bf16, kind="ExternalInput").ap()
    mA = nc.dram_tensor("mA", [P, CA + BLK], bf16, kind="ExternalInput").ap()
    mB = nc.dram_tensor("mB", [P, CB + BLK], bf16, kind="ExternalInput").ap()
    rec_dram = nc.dram_tensor("rec_int", [2, BLK], f32).ap()
    outT = nc.dram_tensor("outT", [C, TL], f32, kind="ExternalOutput").ap()

    aps = (xT, xq, Wk, Wq, Wv, Wo, bq_t, bk_t, bo_t, ones_d, mA, mB,
           rec_dram, outT)
    with tile.TileContext(nc) as tc:
        _emit(nc, tc, aps, T, C)
    nc.compile()
    return nc


def make_core_inputs(x, Wq, bq, Wk, bk, Wv, bv, Wo, bo, T=T_, C=C_):
    """Per-core input maps (list of 8 dicts) for the SPMD program."""
    import ml_dtypes

    f = np.float32
    bf = ml_dtypes.bfloat16
    NT = C // P
    BLK = T // 4
    njA = (2 * BLK) // P
    njB = (4 * BLK) // P
    CA = P * (njA - 1)
    CB = P * (njB - 1 - njA)

    x = np.asarray(x, f)
    Wq, Wk, Wv, Wo = (np.asarray(w, f) for w in (Wq, Wk, Wv, Wo))
    bq, bk, bv, bo = (np.asarray(b, f) for b in (bq, bk, bv, bo))

    def panels(W):  # [C, C] -> [P, co, ci, m]: W[ci*P+p, co*P+m]
        return np.ascontiguousarray(
            W.reshape(NT, P, NT, P).transpose(1, 2, 0, 3)
        ).astype(bf)

    Wk_t = panels(Wk)
    Wq_t = panels(Wq)
    Wo_t = panels(Wo)
    # [C, C] -> [P, ci, m]: Wv[ci*P+p, m]
    Wv_t = np.ascontiguousarray(
        Wv.reshape(NT, P, C).transpose(1, 0, 2)
    ).astype(bf)
    bo_eff = (bv @ Wo + bo).astype(f)

    def tr(b):  # [C] -> [P, NT] with b_t[p, t] = b[t*P + p]
        return np.ascontiguousarray(b.reshape(NT, P).T)

    def mask(CC, i0, width):
        pp = np.arange(P, dtype=np.int64)[:, None]
        gg = np.arange(width, dtype=np.int64)[None, :]
        return np.ascontiguousarray((pp <= gg - CC + i0).astype(bf))

    ones = np.ones((P, 1), bf)

    maps = []
    for core in range(8):
        b, p = core // 2, core % 2
        lo, hi = (0, 3) if p == 0 else (1, 2)
        # [P, chunk, ci, t'] = x[b, chunk*BLK+t', ci*P+p]
        xTv = np.ascontiguousarray(
            x[b].reshape(4, BLK, NT, P).transpose(3, 0, 2, 1)
        ).astype(bf)
        xhalf = np.ascontiguousarray(xTv[:, [2 * p, 2 * p + 1]])
        xqb = np.ascontiguousarray(xTv[:, [lo, hi]])
        maps.append(
            {
                "xT": xhalf,
                "xq": xqb,
                "Wk": Wk_t,
                "Wq": Wq_t,
                "Wv": Wv_t,
                "Wo": Wo_t,
                "bq_t": tr(bq),
                "bk_t": tr(bk),
                "bo_t": tr(bo_eff),
                "ones_d": ones,
                "mA": mask(CA, lo * BLK, CA + BLK),
                "mB": mask(CB + njA * P, hi * BLK, CB + BLK),
            }
        )
    return maps


def gather_output(results, T=T_, C=C_, B=B_):
    BLK = T // 4
    out = np.empty((B, T, C), np.float32)
    for core in range(8):
        b, p = core // 2, core % 2
        lo, hi = (0, 3) if p == 0 else (1, 2)
        oT = results[core]["outT"]
        out[b, lo * BLK:(lo + 1) * BLK] = oT[:, 0:BLK].T
        out[b, hi * BLK:(hi + 1) * BLK] = oT[:, BLK:2 * BLK].T
    return out


_NC_CACHE = {}


def kernel(x, Wq, bq, Wk, bk, Wv, bv, Wo, bo):
    from concourse.bass_utils import run_bass_kernel_spmd

    key = "full"
    if key not in _NC_CACHE:
        _NC_CACHE[key] = build_program()
    nc = _NC_CACHE[key]
    in_maps = make_core_inputs(x, Wq, bq, Wk, bk, Wv, bv, Wo, bo)
    res = run_bass_kernel_spmd(nc, in_maps, list(range(8))).results
    return gather_output(res)


# revision 16
# speedup vs baseline: 1.3213x; 1.2125x over previous
"""Causal full-d_model attention (B=4, T=2048, C=1024) on 8 Trainium2 cores.

Sharding: core = 2*b + p handles batch b and two 512-row sequence blocks,
p=0 -> blocks {0, 3}, p=1 -> blocks {1, 2} (pairing balances causal work).
K/V projections are split across the pair: each core projects its own
1024-token half (per-core input data places the right half in the same
program slot), then an intra-pair AllGather ([[0,1],[2,3],[4,5],[6,7]] -
the valid LNC1_4x2 shape) assembles the full kT / v, read back into SBUF
in global token order.  This halves the projection matmul work vs
computing K/V redundantly on both cores.

All matmul operands are bf16 (same PE rate as f32r, but FWL halves
LDWEIGHTS and every DMA byte count halves).  Accumulation is fp32 in
PSUM; softmax denominator / reciprocal / biases stay fp32.

On-device layout is transposed ([feature, token]) so every matmul
contracts along the partition axis:
    kT/qT = W.T @ xT            (projection)
    v     = xT.T @ Wv           (token-partition layout)
    scoresT[j, i] = kT_j.T @ qT (j on partitions)
    attnT[c, i]  += v_j.T @ probsT
    outT          = Wo_slice.T @ attnT
Softmax is unnormalized (scores ~ N(0,1), exp is safe); the denominator
comes from an M=1 ones-column matmul over masked exp tiles and is applied
at the PSUM->SBUF copy via a DRAM-broadcast reciprocal row.  Causal masks
arrive as per-core input data, so all 8 cores run one SPMD program.

Input loads ride the sync HWDGE FIFO in exact need-order (first matmul
needs only ~1.3 MB); the AllGather staging/readback path uses the gpsimd
queue so it never blocks input streaming.  Phase-2 emission order hides
the reciprocal round-trip (scores-B between PV-A and O-A) and PV
accumulates in two 4-bank PSUM groups to fit the 8 banks.
"""

import math

import numpy as np

P = 128          # SBUF partitions
B_, T_, C_ = 4, 2048, 1024
RG = [[0, 1], [2, 3], [4, 5], [6, 7]]   # intra-pair replica groups


def _emit(nc, tc, aps, T, C):
    import concourse.bass as bass
    from concourse import mybir
    from concourse.tile_rust import add_dep_helper
    from contextlib import ExitStack

    AFT = mybir.ActivationFunctionType
    f32 = mybir.dt.float32
    bf16 = mybir.dt.bfloat16

    NT = C // P            # feature tiles (8)
    BLK = T // 4           # sequence block = i-slot width FB (512)
    TL = 2 * BLK           # local query tokens per core
    TH = T // 2            # K/V tokens projected per core
    FB = BLK               # matmul moving free dim
    NCHL = TH // BLK       # local K/V chunks (2)
    njA = (2 * BLK) // P   # 8  j-tiles for slot A
    njB = (4 * BLK) // P   # 16 j-tiles for slot B
    CA = P * (njA - 1)
    NJB0 = njA             # slot-B j-tiles < NJB0 are causally all-ones
    CB = P * (njB - 1 - NJB0)
    SC = 1.0 / math.sqrt(C)

    (xT, xq, Wk, Wq, Wv, Wo, bq_t, bk_t, bo_t, ones_d, mA, mB,
     rec_dram, outT) = aps

    with ExitStack() as ctx:
        singles = ctx.enter_context(tc.tile_pool(name="singles", bufs=1))
        kpool = ctx.enter_context(tc.tile_pool(name="kpool", bufs=1))
        qpool = ctx.enter_context(tc.tile_pool(name="qpool", bufs=1))
        vpool = ctx.enter_context(tc.tile_pool(name="vpool", bufs=1))
        wopool = ctx.enter_context(tc.tile_pool(name="wopool", bufs=1))
        psp = ctx.enter_context(tc.tile_pool(name="psp", bufs=8, space="PSUM"))
        dramp = ctx.enter_context(tc.tile_pool(name="dramp", bufs=1,
                                               space="DRAM"))

        bq_sb = singles.tile([P, NT], f32, name="bq_sb")
        bk_sb = singles.tile([P, NT], f32, name="bk_sb")
        bo_sb = singles.tile([P, NT], f32, name="bo_sb")
        ones_sb = singles.tile([P, 1], bf16, name="ones_sb")
        mA_sb = singles.tile([P, CA + FB], bf16, name="mA_sb")
        mB_sb = singles.tile([P, CB + FB], bf16, name="mB_sb")
        nc.sync.dma_start(out=bq_sb, in_=bq_t)
        nc.sync.dma_start(out=bk_sb, in_=bk_t)
        nc.sync.dma_start(out=bo_sb, in_=bo_t)
        nc.sync.dma_start(out=ones_sb, in_=ones_d)

        kT_sb = kpool.tile([P, NT, T], bf16, name="kT_sb")
        qT_sb = qpool.tile([P, NT, TL], bf16, name="qT_sb")
        v_sb = vpool.tile([P, T // P, C], bf16, name="v_sb")
        wo_sb = wopool.tile([P, NT, NT, P], bf16, name="wo_sb")

        # DRAM bounce buffers for the intra-pair AllGather
        kag_in = dramp.tile([P, NT, TH], bf16, name="kag_in")
        kag_out = dramp.tile([2, P, NT, TH], bf16, name="kag_out")
        vag_in = dramp.tile([P, TH // P, C], bf16, name="vag_in")
        vag_out = dramp.tile([2, P, TH // P, C], bf16, name="vag_out")

        # ------- phase 1: K-half, V-half (+AllGathers), then Q -------
        with ExitStack() as p1:
            wpool = p1.enter_context(tc.tile_pool(name="wpool", bufs=1))
            xcpool = p1.enter_context(tc.tile_pool(name="xcpool", bufs=1))
            xqpool = p1.enter_context(tc.tile_pool(name="xqpool", bufs=1))
            kstp = p1.enter_context(tc.tile_pool(name="kstp", bufs=2))
            vstp = p1.enter_context(tc.tile_pool(name="vstp", bufs=4))

            wk_sb = wpool.tile([P, NT, NT, P], bf16, name="wk_sb")
            wq_sb = wpool.tile([P, NT, NT, P], bf16, name="wq_sb")
            wv_sb = wpool.tile([P, NT, C], bf16, name="wv_sb")
            xc_sb = xcpool.tile([P, NCHL, NT, BLK], bf16, name="xc_sb")
            xq_sb = xqpool.tile([P, 2, NT, FB], bf16, name="xq_sb")

            # input loads on the sync FIFO in exact need-order
            nc.sync.dma_start(out=wk_sb[:, 0], in_=Wk[:, 0])
            nc.sync.dma_start(out=xc_sb[:, 0], in_=xT[:, 0])
            for co in range(1, NT):
                nc.sync.dma_start(out=wk_sb[:, co], in_=Wk[:, co])
            nc.sync.dma_start(out=xc_sb[:, 1], in_=xT[:, 1])
            nc.sync.dma_start(out=wv_sb, in_=Wv)
            nc.sync.dma_start(out=xq_sb, in_=xq)
            for co in range(NT):
                nc.sync.dma_start(out=wq_sb[:, co], in_=Wq[:, co])
            nc.sync.dma_start(out=mA_sb, in_=mA)
            nc.sync.dma_start(out=mB_sb, in_=mB)
            nc.sync.dma_start(out=wo_sb, in_=Wo)

            # K-half: kT_loc = Wk.T @ x_half (+bk), staged to DRAM per
            # panel so the AllGather can trigger as early as possible
            for l in range(NCHL):
                kst = kstp.tile([P, NT, BLK], bf16, name="kst", tag="kst")
                for co in range(NT):
                    ps = psp.tile([P, BLK], f32, name="ps_k", tag="ps")
                    for ci in range(NT):
                        nc.tensor.matmul(
                            ps,
                            wk_sb[:, co, ci, :],
                            xc_sb[:, l, ci, :],
                            start=(ci == 0),
                            stop=(ci == NT - 1),
                        )
                    nc.scalar.activation(
                        out=kst[:, co, :], in_=ps, func=AFT.Identity,
                        bias=bk_sb[:, co:co + 1],
                    )
                    nc.gpsimd.dma_start(
                        out=kag_in[:, co, l * BLK:(l + 1) * BLK],
                        in_=kst[:, co, :],
                    )
            nc.gpsimd.collective_compute(
                "AllGather",
                mybir.AluOpType.bypass,
                replica_groups=RG,
                ins=[kag_in.opt()],
                outs=[kag_out.opt()],
            )

            # V-half: v_loc = x_half @ Wv, staged to DRAM
            # (bv is folded into bo_t on the host)
            for l in range(NCHL):
                for jt in range(BLK // P):
                    for ch in range(C // 512):
                        ps = psp.tile([P, 512], f32, name="ps_v", tag="ps")
                        for ci in range(NT):
                            nc.tensor.matmul(
                                ps,
                                xc_sb[:, l, ci, jt * P:(jt + 1) * P],
                                wv_sb[:, ci, ch * 512:(ch + 1) * 512],
                                start=(ci == 0),
                                stop=(ci == NT - 1),
                            )
                        vs = vstp.tile([P, 512], bf16, name="vs", tag="vs")
                        nc.vector.tensor_copy(vs, ps)
                        nc.gpsimd.dma_start(
                            out=vag_in[:, l * (BLK // P) + jt,
                                       ch * 512:(ch + 1) * 512],
                            in_=vs,
                        )
            nc.gpsimd.collective_compute(
                "AllGather",
                mybir.AluOpType.bypass,
                replica_groups=RG,
                ins=[vag_in.opt()],
                outs=[vag_out.opt()],
            )

            # readbacks in global token order (h = rank in pair); on the
            # sync queue so they never delay the AllGather triggers
            for h in range(2):
                nc.sync.dma_start(
                    out=kT_sb[:, :, h * TH:(h + 1) * TH], in_=kag_out[h]
                )
            for h in range(2):
                nc.sync.dma_start(
                    out=v_sb[:, h * (TH // P):(h + 1) * (TH // P), :],
                    in_=vag_out[h],
                )

            # Q: qT = Wq.T @ xq (+bq) for the two local 512-blocks
            for s in range(2):
                for co in range(NT):
                    ps = psp.tile([P, FB], f32, name="ps_q", tag="ps")
                    for ci in range(NT):
                        nc.tensor.matmul(
                            ps,
                            wq_sb[:, co, ci, :],
                            xq_sb[:, s, ci, :],
                            start=(ci == 0),
                            stop=(ci == NT - 1),
                        )
                    nc.scalar.activation(
                        out=qT_sb[:, co, s * FB:(s + 1) * FB],
                        in_=ps,
                        func=AFT.Identity,
                        bias=bq_sb[:, co:co + 1],
                    )

        # -------- phase 2: attention + output projection --------
        with ExitStack() as p2:
            probsp = p2.enter_context(tc.tile_pool(name="probsp",
                                                   bufs=njA + njB))
            attnp = p2.enter_context(tc.tile_pool(name="attnp", bufs=2))
            recp = p2.enter_context(tc.tile_pool(name="recp", bufs=2))
            ostagep = p2.enter_context(tc.tile_pool(name="ostagep", bufs=2))

            SLOTS = [(njA, 0, CA, mA_sb), (njB, NJB0, CB, mB_sb)]

            def emit_scores(a):
                nj, j0m, Cm, m_sb = SLOTS[a]
                pjs = []
                ps_den = psp.tile([1, FB], f32, name="ps_den", tag="ps")
                for jt in range(nj):
                    ps_s = psp.tile([P, FB], f32, name="ps_s", tag="ps")
                    for ci in range(NT):
                        nc.tensor.matmul(
                            ps_s,
                            kT_sb[:, ci, jt * P:(jt + 1) * P],
                            qT_sb[:, ci, a * FB:(a + 1) * FB],
                            start=(ci == 0),
                            stop=(ci == NT - 1),
                        )
                    pj = probsp.tile([P, FB], bf16, name="pj", tag="pj")
                    nc.scalar.activation(out=pj, in_=ps_s, func=AFT.Exp, scale=SC)
                    if jt >= j0m:  # earlier j-tiles are all-ones on every core
                        s0 = Cm - P * (jt - j0m)
                        nc.vector.tensor_mul(pj, pj, m_sb[:, s0:s0 + FB])
                    nc.tensor.matmul(
                        ps_den,
                        ones_sb,
                        pj,
                        start=(jt == 0),
                        stop=(jt == nj - 1),
                        skip_group_check=True,
                    )
                    pjs.append(pj)
                # 1/denominator: quick copy releases the PSUM bank, then the
                # slow reciprocal runs off the SBUF copy; broadcast to 128
                # partitions via a stride-0 DRAM read.
                den_sb = recp.tile([1, FB], f32, name="den_sb", tag="den_sb")
                nc.scalar.copy(den_sb, ps_den)
                rrow = recp.tile([1, FB], f32, name="rrow", tag="rrow")
                nc.vector.reciprocal(rrow, den_sb)
                rec_w = nc.sync.dma_start(out=rec_dram[a:a + 1, :], in_=rrow)
                recipB = recp.tile([P, FB], f32, name="recipB", tag="recipB")
                rec_row = rec_dram[a, :]
                rec_bcast = bass.AP(
                    tensor=rec_row.tensor,
                    offset=rec_row.offset,
                    ap=[[0, P]] + [list(d) for d in rec_row.ap],
                )
                rec_r = nc.sync.dma_start(out=recipB, in_=rec_bcast)
                add_dep_helper(rec_r.ins, rec_w.ins, reason="rec_dram RAW")
                return pjs, recipB

            def emit_pv(a, pjs, recipB):
                # PV in two 4-bank PSUM groups so scores of the next slot
                # can run while the first group's banks drain
                nj = SLOTS[a][0]
                attn_sb = attnp.tile([P, NT, FB], bf16, name="attn_sb",
                                     tag="attn")
                for g0 in (0, NT // 2):
                    ps_attn = [
                        psp.tile([P, FB], f32, name="ps_attn", tag="ps")
                        for _ in range(NT // 2)
                    ]
                    for jt in range(nj):
                        for k, ct in enumerate(range(g0, g0 + NT // 2)):
                            nc.tensor.matmul(
                                ps_attn[k],
                                v_sb[:, jt, ct * P:(ct + 1) * P],
                                pjs[jt],
                                start=(jt == 0),
                                stop=(jt == nj - 1),
                                skip_group_check=True,
                            )
                    for k, ct in enumerate(range(g0, g0 + NT // 2)):
                        nc.vector.tensor_mul(
                            attn_sb[:, ct, :], ps_attn[k], recipB
                        )
                return attn_sb

            def emit_oproj(a, attn_sb):
                for co in range(NT):
                    ps_o = psp.tile([P, FB], f32, name="ps_o", tag="ps")
                    for ci in range(NT):
                        nc.tensor.matmul(
                            ps_o,
                            wo_sb[:, co, ci, :],
                            attn_sb[:, ci, :],
                            start=(ci == 0),
                            stop=(ci == NT - 1),
                        )
                    os_ = ostagep.tile([P, FB], f32, name="os_", tag="os")
                    nc.scalar.activation(
                        out=os_, in_=ps_o, func=AFT.Identity,
                        bias=bo_sb[:, co:co + 1],
                    )
                    nc.sync.dma_start(
                        out=outT[co * P:(co + 1) * P, a * FB:(a + 1) * FB],
                        in_=os_,
                    )

            # Both score passes run before any PV: scores only need kT
            # (the first AllGather), so the v AllGather + readback gets
            # ~28us more compute to hide under before PV-A reads v.
            pjs_A, recB_A = emit_scores(0)
            pjs_B, recB_B = emit_scores(1)
            attn_A = emit_pv(0, pjs_A, recB_A)
            emit_oproj(0, attn_A)
            attn_B = emit_pv(1, pjs_B, recB_B)
            emit_oproj(1, attn_B)


def build_program(T=T_, C=C_, num_cores=8):
    """Build and compile the SPMD Bass program."""
    from concourse import bacc, mybir
    import concourse.tile as tile

    f32 = mybir.dt.float32
    bf16 = mybir.dt.bfloat16
    NT = C // P
    BLK = T // 4
    TL = 2 * BLK
    njA = (2 * BLK) // P
    njB = (4 * BLK) // P
    CA = P * (njA - 1)
    CB = P * (njB - 1 - njA)

    nc = bacc.Bacc(
        "TRN2", target_bir_lowering=False, debug=False, num_devices=num_cores
    )
    xT = nc.dram_tensor("xT", [P, 2, NT, BLK], bf16, kind="ExternalInput").ap()
    xq = nc.dram_tensor("xq", [P, 2, NT, BLK], bf16, kind="ExternalInput").ap()
    Wk = nc.dram_tensor("Wk", [P, NT, NT, P], bf16, kind="ExternalInput").ap()
    Wq = nc.dram_tensor("Wq", [P, NT, NT, P], bf16, kind="ExternalInput").ap()
    Wv = nc.dram_tensor("Wv", [P, NT, C], bf16, kind="ExternalInput").ap()
    Wo = nc.dram_tensor("Wo", [P, NT, NT, P], bf16, kind="ExternalInput").ap()
    bq_t = nc.dram_tensor("bq_t", [P, NT], f32, kind="ExternalInput").ap()
    bk_t = nc.dram_tensor("bk_t", [P, NT], f32, kind="ExternalInput").ap()
    bo_t = nc.dram_tensor("bo_t", [P, NT], f32, kind="ExternalInput").ap()
    ones_d = nc.dram_tensor("ones_d", [P, 1], bf16, kind="ExternalInput").ap()
    mA = nc.dram_tensor("mA", [P, CA + BLK], bf16, kind="ExternalInput").ap()
    mB = nc.dram_tensor("mB", [P, CB + BLK], bf16, kind="ExternalInput").ap()
    rec_dram = nc.dram_tensor("rec_int", [2, BLK], f32).ap()
    outT = nc.dram_tensor("outT", [C, TL], f32, kind="ExternalOutput").ap()

    aps = (xT, xq, Wk, Wq, Wv, Wo, bq_t, bk_t, bo_t, ones_d, mA, mB,
           rec_dram, outT)
    with tile.TileContext(nc) as tc:
        _emit(nc, tc, aps, T, C)
    nc.compile()
    return nc


def make_core_inputs(x, Wq, bq, Wk, bk, Wv, bv, Wo, bo, T=T_, C=C_):
    """Per-core input maps (list of 8 dicts) for the SPMD program."""
    import ml_dtypes

    f = np.float32
    bf = ml_dtypes.bfloat16
    NT = C // P
    BLK = T // 4
    njA = (2 * BLK) // P
    njB = (4 * BLK) // P
    CA = P * (njA - 1)
    CB = P * (njB - 1 - njA)

    x = np.asarray(x, f)
    Wq, Wk, Wv, Wo = (np.asarray(w, f) for w in (Wq, Wk, Wv, Wo))
    bq, bk, bv, bo = (np.asarray(b, f) for b in (bq, bk, bv, bo))

    def panels(W):  # [C, C] -> [P, co, ci, m]: W[ci*P+p, co*P+m]
        return np.ascontiguousarray(
            W.reshape(NT, P, NT, P).transpose(1, 2, 0, 3)
        ).astype(bf)

    Wk_t = panels(Wk)
    Wq_t = panels(Wq)
    Wo_t = panels(Wo)
    # [C, C] -> [P, ci, m]: Wv[ci*P+p, m]
    Wv_t = np.ascontiguousarray(
        Wv.reshape(NT, P, C).transpose(1, 0, 2)
    ).astype(bf)
    bo_eff = (bv @ Wo + bo).astype(f)

    def tr(b):  # [C] -> [P, NT] with b_t[p, t] = b[t*P + p]
        return np.ascontiguousarray(b.reshape(NT, P).T)

    def mask(CC, i0, width):
        pp = np.arange(P, dtype=np.int64)[:, None]
        gg = np.arange(width, dtype=np.int64)[None, :]
        return np.ascontiguousarray((pp <= gg - CC + i0).astype(bf))

    ones = np.ones((P, 1), bf)

    maps = []
    for core in range(8):
        b, p = core // 2, core % 2
        lo, hi = (0, 3) if p == 0 else (1, 2)
        # [P, chunk, ci, t'] = x[b, chunk*BLK+t', ci*P+p]
        xTv = np.ascontiguousarray(
            x[b].reshape(4, BLK, NT, P).transpose(3, 0, 2, 1)
        ).astype(bf)
        xhalf = np.ascontiguousarray(xTv[:, [2 * p, 2 * p + 1]])
        xqb = np.ascontiguousarray(xTv[:, [lo, hi]])
        maps.append(
            {
                "xT": xhalf,
                "xq": xqb,
                "Wk": Wk_t,
                "Wq": Wq_t,
                "Wv": Wv_t,
                "Wo": Wo_t,
                "bq_t": tr(bq),
                "bk_t": tr(bk),
                "bo_t": tr(bo_eff),
                "ones_d": ones,
                "mA": mask(CA, lo * BLK, CA + BLK),
                "mB": mask(CB + njA * P, hi * BLK, CB + BLK),
            }
        )
    return maps


def gather_output(results, T=T_, C=C_, B=B_):
    BLK = T // 4
    out = np.empty((B, T, C), np.float32)
    for core in range(8):
        b, p = core // 2, core % 2
        lo, hi = (0, 3) if p == 0 else (1, 2)
        oT = results[core]["outT"]
        out[b, lo * BLK:(lo + 1) * BLK] = oT[:, 0:BLK].T
        out[b, hi * BLK:(hi + 1) * BLK] = oT[:, BLK:2 * BLK].T
    return out


_NC_CACHE = {}


def kernel(x, Wq, bq, Wk, bk, Wv, bv, Wo, bo):
    from concourse.bass_utils import run_bass_kernel_spmd

    key = "full"
    if key not in _NC_CACHE:
        _NC_CACHE[key] = build_program()
    nc = _NC_CACHE[key]
    in_maps = make_core_inputs(x, Wq, bq, Wk, bk, Wv, bv, Wo, bo)
    res = run_bass_kernel_spmd(nc, in_maps, list(range(8))).results
    return gather_output(res)
